# revision 46
# baseline (speedup 1.0000x reference)
"""Distributed GQA attention (B=2,T=2048,C=2048,H=16,KV=4,D=128, RoPE, causal)
for one TRN2 chip (8 NeuronCores).

Sharding (no collectives except KV AllGather): core c -> batch b=c//4,
stripe s=c%4. Each core handles query rows {r : r % 4 == s} of its batch
(512 rows, interleaved so causal spans are shape-uniform across cores ->
one SPMD graph), computes K/V for its 512-token chunk (KV proj sharded,
AllGather within the 4-core batch group), and produces complete output
rows. Host reassembles by stripe.

Scheduling model (from perfetto forensics): ONE dynamic DMA engine
(~200GB/s) serves the sync/scalar/gpsimd queues round-robin and crawls to
~50-70GB/s while the AllGather's CC engines run; the PE sustains ~1 bf16
col / 0.6ns (power-throttle duty ~69%) making total matmul columns the
hard floor (~285us); the AllGather (barrier 15-70us + transfer 42-75us)
is high-variance and must stay off the critical path.

Per-core pipeline:
  PE-warmup dummy matmuls at t~0 (p-state) while the sync queue pulls, in
    need-order: KV operands -> rope-k tables -> xq -> 10 Wq head-tiles.
    Scalar queue stays empty so the incremental K/V staging descriptors
    fire the moment RoPE/V-copy finish (~55us); deferrable loads (rope-q
    tables, mask, late Wq heads) queue behind them to ride the crawl.
  KVproj (ct-outer, K-banks/V-banks in separate PSUM pools so Qproj's
    accumulators reuse the earliest-released banks) -> RoPE on DVE ->
    stage -> AllGather; Wo loads ride gpsimd after the V readbacks into
    the SBUF region freed by the KV pool (no pool-close barrier on the
    Qproj/attention path waits on staging or Wo).
  Qproj overlaps the AllGather; K readbacks on sync / V on gpsimd, rank 0
    first, so attention starts the moment rank-0 data lands.
  attention, scores transposed: S^T[k,(h4,q)] = K-tile^T . Q(4 heads)
    + staircase causal mask (DVE), exp on ACT -> P^T bf16; softmax
    denominator is a bf16 running sum of P^T tiles + ONE ones-vector
    matmul per (qt,g); staircase tiles use partial-free APs.
  Oproj chains are generators pumped one matmul at a time between
    attention matmuls (PE filler while ACT paces the exp chain); outputs
    drain on the (late-idle) sync queue in bf16.
"""

import numpy as np
import ml_dtypes

import concourse.bass as bass
import concourse.tile as tile
from concourse import bacc, mybir
from concourse.bass_utils import run_bass_kernel_spmd

B, T, C = 2, 2048, 2048
H, KV, D = 16, 4, 128
G4 = H // KV            # q heads per kv head
THETA = 10000.0
P = 128
CT = C // P             # 16 c-tiles
TQ = 512                # queries per core
NQT = TQ // P           # 4 q-tiles
NTT = T // P            # 16 token tiles
NCC = C // 512          # 4 output column blocks
MASK_VAL = -1e5

f32 = mybir.dt.float32
bf16 = mybir.dt.bfloat16

_compiled = {}


def _build():
    nc = bacc.Bacc("TRN2", target_bir_lowering=False, debug=False, num_devices=8)
    xq_e = nc.dram_tensor("xq", [P, CT * TQ], bf16, kind="ExternalInput")
    xkv_e = nc.dram_tensor("xkv", [P, CT * TQ], bf16, kind="ExternalInput")  # chunk, pre-tiled [p,(ct t)]
    wq_e = nc.dram_tensor("wq", [P, H * CT * D], bf16, kind="ExternalInput")  # [p,(h ct d)]
    wkv_e = nc.dram_tensor("wkv", [P, CT * 2 * KV * D], bf16, kind="ExternalInput")  # [p,(ct n)]
    wo_e = nc.dram_tensor("wo", [P, NCC * H * 512], bf16, kind="ExternalInput")  # [p,(cc hh c)]
    cq_e = nc.dram_tensor("cos_q", [D, TQ], bf16, kind="ExternalInput")
    sq_e = nc.dram_tensor("sin_q", [D, TQ], bf16, kind="ExternalInput")
    ck_e = nc.dram_tensor("cos_k", [D, TQ], bf16, kind="ExternalInput")  # chunk positions
    sk_e = nc.dram_tensor("sin_k", [D, TQ], bf16, kind="ExternalInput")
    mk_e = nc.dram_tensor("mask", [P, NQT * P], bf16, kind="ExternalInput")
    out_e = nc.dram_tensor("out", [TQ, C], bf16, kind="ExternalOutput")

    NR = 4  # ranks per batch group

    from contextlib import ExitStack  # noqa: F401

    with tile.TileContext(nc) as tc, ExitStack() as top:
        persist = top.enter_context(tc.tile_pool(name="persist", bufs=1))

        # (A warmup barrier collective was tried and removed: the CC stream's
        # ~45us first-collective rendezvous runs concurrently with KVproj
        # regardless, and each extra CC op adds ~10us of stream turnaround
        # that pushed the real AllGather later.)
        mask_t = persist.tile([P, NQT, P], bf16)
        ones_col = persist.tile([P, 1], bf16)
        nc.vector.memset(ones_col[:], 1.0)
        # PE warmup: ~2.5us of dummy matmuls at t~0 (before any DMA lands)
        # so the PE's p-state/clock is up when the first real KV chain start.
        with tc.tile_pool(name="warm", bufs=1) as warmp, \
             tc.tile_pool(name="ps_warm", bufs=1, space="PSUM") as ps_warmp:
            wsb = warmp.tile([P, TQ], bf16)
            nc.vector.memset(wsb[:], 0.0)
            ps_w = ps_warmp.tile([1, TQ], f32)
            for i in range(8):
                nc.tensor.matmul(ps_w[:], ones_col[:], wsb[:],
                                 start=(i == 0), stop=(i == 7))
        # qhat laid out [d, g, q, j] so the scores moving operand (q-major,
        # head-minor) is contiguous.
        qhat = persist.tile([D, KV, TQ, G4], bf16)
        # per-rank K/V tiles: attention q-tile 0 only depends on rank 0's
        # chunk, so it starts as soon as the first readback DMA lands.
        khat_r = [persist.tile([D, KV, TQ], bf16, name=f"khat{r}") for r in range(NR)]
        vsb_r = [persist.tile([P, NQT, KV * D], bf16, name=f"vsb{r}") for r in range(NR)]
        yhat = persist.tile([D, H, TQ], bf16)

        # Q operand pools (DMAs on the scalar queue so the sync queue stays
        # clear for the collective staging + readback). Kept open to the end
        # (LIFO pool order: wop is created after the KV pool closes so it can
        # reuse that region, and must outlive these).
        xqp = top.enter_context(tc.tile_pool(name="xqp", bufs=1))
        tabq = top.enter_context(tc.tile_pool(name="tabq", bufs=1))
        # Wq stream pool coexists with the KV pool: 9 head-tiles prefetch
        # with no SBUF anti-dependency, deep enough that Qproj never starves
        # while the AllGather window throttles concurrent DMA to ~60GB/s.
        wstream = top.enter_context(tc.tile_pool(name="wqstream", bufs=9))

        # ---- KV chunk projection (bf16) + AllGather ---------------------
        with tc.tile_pool(name="kvchunk", bufs=1) as kvc, \
             tc.tile_pool(name="dram", bufs=1, space="DRAM") as dram, \
             tc.tile_pool(name="ps_kvk", bufs=1, space="PSUM") as ps_kvk, \
             tc.tile_pool(name="ps_kvv", bufs=1, space="PSUM") as ps_kvv, \
             tc.tile_pool(name="ropek", bufs=1) as ropekp:
            # per-c-tile tiles so the first KV matmuls start as soon as the
            # first chunks land (exact per-tile deps). ct-outer loop with all
            # 8 K/V accumulators resident (8 PSUM banks) -> the tensor engine
            # starts ~1us in and streams DMA-paced, instead of waiting for
            # the full x-chunk before the first chain.
            # 4-ct granularity: 8 big DMAs instead of 32 small ones — the SP
            # sequencer's ~0.6us per-issue cost was pacing the startup.
            xkv_r = xkv_e.ap().rearrange("p (cg ct t) -> p cg ct t", cg=4, ct=4)
            wkv_r = wkv_e.ap().rearrange("p (cg ct n) -> p cg ct n", cg=4, ct=4)
            # first 4 c-tiles as separate small tiles (deps are per-tile, so
            # the first chains start ~1us in); the rest in 512KB groups.
            # Weight tile first within each pair (LDWEIGHTS consumes it first).
            x0s, w0s = [], []
            for ci in range(4):
                wt = kvc.tile([P, 2 * KV * D], bf16, name=f"wkv0_{ci}")
                nc.sync.dma_start(wt[:], wkv_r[:, 0, ci])
                w0s.append(wt)
                xt = kvc.tile([P, TQ], bf16, name=f"xkv0_{ci}")
                nc.sync.dma_start(xt[:], xkv_r[:, 0, ci])
                x0s.append(xt)
            # All KV operands land BEFORE xq: KVproj-end gates the staging ->
            # AllGather -> readback chain (the critical path to attention),
            # while Qproj only starts once the KV chains free the PE (~46us),
            # well after even a late xq.
            xkv4, wkv4 = [None], [None]
            for cg in range(1, 4):
                wt = kvc.tile([P, 4, 2 * KV * D], bf16, name=f"wkv{cg}")
                nc.sync.dma_start(wt[:], wkv_r[:, cg])
                wkv4.append(wt)
                xt = kvc.tile([P, 4, TQ], bf16, name=f"xkv{cg}")
                nc.sync.dma_start(xt[:], xkv_r[:, cg])
                xkv4.append(xt)
            # xq + the first 8 Wq head-tiles follow on the SAME sync queue:
            # one queue pulling serially gets full HBM rate, so the KV
            # operands (which gate staging -> AllGather, the critical path)
            # land first and fast; xq/wq ride the remaining pre-staging window
            # (staging waits on the K RoPE ~50us anyway). The late wq tiles
            # (h8-15) stream on the scalar queue during the AllGather window.
            # ONE DMA engine (~200GB/s) serves all three dynamic queues
            # round-robin, and it crawls (~50GB/s) while the AllGather's CC
            # engines run. So: every byte needed before/at the AllGather is
            # serialized on the sync queue in need-order (KV -> rope-k ->
            # xq -> wq h0..h7), the scalar queue stays EMPTY so the staging
            # descriptors sit at its head and fire the moment RoPE finishes,
            # and everything deferrable (mask, rope-q tables, wq h8-15)
            # queues on scalar BEHIND staging to ride the crawl window.
            cos_k = kvc.tile([D, TQ], bf16)
            nc.sync.dma_start(cos_k[:], ck_e.ap())
            sin_k = kvc.tile([D, TQ], bf16)
            nc.sync.dma_start(sin_k[:], sk_e.ap())
            xq = xqp.tile([P, CT, TQ], bf16)
            xq_r = xq_e.ap().rearrange("p (cg ct q) -> p cg ct q", cg=4, ct=4)
            for cg in range(4):
                nc.sync.dma_start(xq[:, 4 * cg:4 * cg + 4, :], xq_r[:, cg])
            wq_rr = wq_e.ap().rearrange("p (h ct d) -> p h ct d", h=H, ct=CT)
            wq_pre = []
            for h in range(10):
                wqt = wstream.tile([P, CT, D], bf16, tag="wq")
                nc.sync.dma_start(wqt[:], wq_rr[:, h])
                wq_pre.append(wqt)

            def xop(ct, lo=0, hi=TQ):
                if ct < 4:
                    return x0s[ct][:, lo:hi]
                return xkv4[ct // 4][:, ct % 4, lo:hi]

            def wop(ct, lo, hi):
                if ct < 4:
                    return w0s[ct][:, lo:hi]
                return wkv4[ct // 4][:, ct % 4, lo:hi]
            # kchunk/vchunk live in persist: staging reads them, and if they
            # were kvc tiles the kvc pool-close barrier (region reused by
            # later pools) would gate Qproj's start on staging completion.
            kchunk = persist.tile([D, KV, TQ], bf16)
            vchunk = persist.tile([P, NQT, KV * D], bf16)

            # ct-outer with all 8 K/V accumulators resident (8 PSUM banks):
            # every chain progresses with the DMA stream and they all finish
            # right as the last c-tile lands. (g-outer K with trailing ropes
            # measured worse: it delays V/staging and Qproj more than the
            # earlier kchunk helps.)
            # K accumulators in the lower PSUM pool, V in the upper: the V
            # pool closes first (LIFO) so Qproj's ps_q banks reuse the V
            # banks, whose consumers (ACT V-copies, ~50us) finish before the
            # rope chain (~55us) that holds the K banks.
            ps_ks = [ps_kvk.tile([P, TQ], f32, name=f"ps_k{g}") for g in range(KV)]
            ps_vs = [ps_kvv.tile([P, KV * D], f32, name=f"ps_v{t}") for t in range(NQT)]
            for ct in range(CT):
                for g in range(KV):
                    nc.tensor.matmul(ps_ks[g][:],
                                     wop(ct, g * D, (g + 1) * D),
                                     xop(ct),
                                     start=(ct == 0), stop=(ct == CT - 1))
                for ttl in range(NQT):
                    nc.tensor.matmul(ps_vs[ttl][:],
                                     xop(ct, ttl * P, (ttl + 1) * P),
                                     wop(ct, KV * D, 2 * KV * D),
                                     start=(ct == 0), stop=(ct == CT - 1))
            # RoPE on DVE; V drains on ACT in parallel. Stage each K group /
            # V tile to DRAM the moment it's ready (incremental staging pulls
            # the AllGather's data-ready point a few us earlier).
            cc_in = dram.tile([2, P, KV, TQ], bf16)
            cc_out = dram.tile([NR, 2, P, KV, TQ], bf16)
            for g in range(KV):
                ps = ps_ks[g]
                tmp = ropekp.tile([D, TQ], bf16, tag="rope_k")
                nc.vector.tensor_copy(tmp[0:64, :], ps[64:128, :])
                nc.vector.tensor_copy(tmp[64:128, :], ps[0:64, :])
                ksl = kchunk[:, g, :]
                nc.vector.tensor_mul(ksl, ps[:], cos_k[:])
                nc.vector.tensor_mul(tmp[:], tmp[:], sin_k[:])
                nc.vector.tensor_add(ksl, ksl, tmp[:])
                nc.scalar.dma_start(cc_in[0, :, g, :], ksl)
            for ttl in range(NQT):
                nc.scalar.activation(vchunk[:, ttl, :], ps_vs[ttl][:],
                                     mybir.ActivationFunctionType.Copy)
                nc.scalar.dma_start(
                    cc_in[1].rearrange("p g x -> p (g x)")[:, ttl * KV * D:(ttl + 1) * KV * D],
                    vchunk[:, ttl, :])
            # deferrable loads queue on scalar BEHIND staging: rope-q tables
            # (the first Qproj rope may lag; attention only needs qhat at
            # ~137us) and the mask (needed at attention start).
            cos_q = tabq.tile([D, TQ], bf16)
            nc.scalar.dma_start(cos_q[:], cq_e.ap())
            sin_q = tabq.tile([D, TQ], bf16)
            nc.scalar.dma_start(sin_q[:], sq_e.ap())
            nc.scalar.dma_start(mask_t[:], mk_e.ap().rearrange("p (kt q) -> p kt q", kt=NQT))
            nc.gpsimd.collective_compute(
                "AllGather",
                mybir.AluOpType.bypass,
                replica_groups=[[0, 1, 2, 3], [4, 5, 6, 7]],
                ins=[cc_in[:].opt()],
                outs=[cc_out[:].opt()],
            )
            # per-rank readbacks, rank 0 first: attention starts on rank 0.
            # K on the sync queue, V on the gpsimd queue -> ~2x readback rate.
            for r in range(NR):
                nc.sync.dma_start(khat_r[r][:], cc_out[r, 0])
                nc.gpsimd.dma_start(
                    vsb_r[r][:].rearrange("p ttl n -> p (ttl n)"),
                    cc_out[r, 1].rearrange("p g x -> p (g x)"))

        # ---- Wo resident: loaded right after the KV section closes so the
        # allocator reuses the freed kvc region (anti-dep ~55us, not Qproj
        # end), on the gpsimd queue behind the V readbacks (lands ~150us,
        # needed ~190us) so no sync/scalar semaphore threshold that gates
        # attention's first matmuls ever covers these 4MB.
        wop = top.enter_context(tc.tile_pool(name="wop", bufs=1))
        wo_r = wo_e.ap().rearrange("p (cc hh c) -> p cc hh c", cc=NCC, hh=H)
        wos = []
        for cc in range(NCC):
            wt = wop.tile([P, H, 512], bf16, name=f"wo{cc}")
            nc.gpsimd.dma_start(wt[:], wo_r[:, cc])
            wos.append(wt)

        # ---- Q projection (bf16, weights streamed via scalar queue) -----
        with tc.tile_pool(name="ps_qp", bufs=3, space="PSUM") as ps_q, \
             tc.tile_pool(name="ropeq", bufs=1) as ropep:
            for h in range(H):
                if h < 10:
                    wqt = wq_pre[h]
                else:
                    wqt = wstream.tile([P, CT, D], bf16, tag="wq")
                    nc.scalar.dma_start(
                        wqt[:], wq_e.ap().rearrange("p (h ct d) -> p h ct d", h=H, ct=CT)[:, h])
                ps = ps_q.tile([P, TQ], f32, tag="ps_q")
                for ct in range(CT):
                    nc.tensor.matmul(ps[:], wqt[:, ct, :], xq[:, ct, :],
                                     start=(ct == 0), stop=(ct == CT - 1))
                tmp = ropep.tile([D, TQ], bf16, tag="rope_q")
                nc.vector.tensor_copy(tmp[0:64, :], ps[64:128, :])
                nc.vector.tensor_copy(tmp[64:128, :], ps[0:64, :])
                qsl = qhat[:, h // G4, :, h % G4]
                nc.vector.tensor_mul(qsl, ps[:], cos_q[:])
                nc.vector.tensor_mul(tmp[:], tmp[:], sin_q[:])
                nc.vector.tensor_add(qsl, qsl, tmp[:])

        # ---- attention (scores transposed) + interleaved Oproj ----------
        with tc.tile_pool(name="ptile", bufs=5) as ptp, \
             tc.tile_pool(name="ptil", bufs=2) as ptilp, \
             tc.tile_pool(name="small", bufs=2) as small, \
             tc.tile_pool(name="outp", bufs=1) as outp, \
             tc.tile_pool(name="ps_s", bufs=3, space="PSUM") as ps_sp, \
             tc.tile_pool(name="ps_y", bufs=2, space="PSUM") as ps_yp, \
             tc.tile_pool(name="ps_den", bufs=1, space="PSUM") as ps_denp, \
             tc.tile_pool(name="ps_o", bufs=2, space="PSUM") as ps_op:
            # Oproj chains are emitted as generators and "pumped" one matmul
            # at a time between attention matmuls: the PE gets filler work
            # whenever the exp chain would otherwise let it idle (keeping the
            # p-state up), and the Oproj phase all but disappears.
            from collections import deque
            pending = deque()

            # Oproj chains are split: heads 0-11 (groups g0-g2) become
            # pump-able right after g2's y-slice, so the PE has filler DURING
            # the same q-tile's last (biggest) group instead of exhausting the
            # previous q-tile's chains early; heads 12-15 follow after g3.
            ps_o_map = {}

            def oproj_part(qt, cc, h_lo, h_hi):
                if (qt, cc) not in ps_o_map:
                    ps_o_map[(qt, cc)] = ps_op.tile([P, 512], f32, tag="ps_o",
                                                    name=f"ps_o_{qt}_{cc}")
                ps_o = ps_o_map[(qt, cc)]
                for hh in range(h_lo, h_hi):
                    nc.tensor.matmul(ps_o[:], yhat[:, hh, qt * P:(qt + 1) * P],
                                     wos[cc][:, hh, :],
                                     start=(hh == 0), stop=(hh == H - 1))
                    yield
                if h_hi == H:
                    ps_o_map.pop((qt, cc))
                    osb = outp.tile([P, 512], bf16, tag="osb")
                    nc.scalar.activation(osb[:], ps_o[:],
                                         mybir.ActivationFunctionType.Copy)
                    # sync queue: idle after the K readbacks, and the SP
                    # engine fires triggers promptly (gpsimd's serialize
                    # behind partition_broadcast, delaying the final drain).
                    nc.sync.dma_start(
                        out_e.ap()[qt * P:(qt + 1) * P, cc * 512:(cc + 1) * 512], osb[:])

            def pump(n):
                for _ in range(n):
                    if not pending:
                        return
                    try:
                        next(pending[0])
                    except StopIteration:
                        pending.popleft()

            for qt in range(NQT):
                nkt = 4 * (qt + 1)
                if qt > 0:
                    for cc in range(NCC):
                        pending.append(oproj_part(qt - 1, cc, 0, H))
                for g in range(KV):
                    # free layout is (q, h4) so staircase partial slices
                    # (q >= q0) are contiguous prefixes -> 2-dim APs.
                    ps_y = ps_yp.tile([P, P, G4], f32, tag="ps_y")
                    ptil = ptilp.tile([P, P, G4], bf16, tag="ptil")

                    def emit_scores(kt):
                        ktl = kt - (nkt - 4)  # staircase index when >= 0
                        q0 = 32 * ktl if ktl > 0 else 0
                        ps_s = ps_sp.tile([P, P, G4], f32, tag="ps_s")
                        nc.tensor.matmul(
                            ps_s[:, q0:, :],
                            khat_r[kt // 4][:, g, (kt % 4) * P:(kt % 4 + 1) * P],
                            qhat[:, g, qt * P + q0:(qt + 1) * P, :],
                            start=True, stop=True)
                        if ktl >= 0:
                            nc.vector.tensor_add(
                                ps_s[:, q0:, :], ps_s[:, q0:, :],
                                mask_t[:, ktl, q0:, None].to_broadcast((P, P - q0, G4)))
                        pt = ptp.tile([P, P, G4], bf16, tag="pt")
                        nc.scalar.activation(pt[:, q0:, :], ps_s[:, q0:, :],
                                             mybir.ActivationFunctionType.Exp)
                        return (kt, q0, pt)

                    def emit_av(kt, q0, pt):
                        nc.tensor.matmul(ps_y[:, q0:, :],
                                         vsb_r[kt // 4][:, kt % 4, g * D:(g + 1) * D],
                                         pt[:, q0:, :],
                                         start=(kt == 0), stop=(kt == nkt - 1))
                        pump(2)
                        # P-tile running sum (softmax denominator), partial
                        # slices on staircase tiles (the skipped prefix is
                        # exactly-masked, i.e. contributes zero), emitted
                        # trailing the exp by the AV lookahead so DVE's
                        # in-order stream never blocks a mask add on an exp.
                        if kt == 0:
                            nc.vector.tensor_copy(ptil[:], pt[:])
                        else:
                            nc.vector.tensor_add(ptil[:, q0:, :], ptil[:, q0:, :],
                                                 pt[:, q0:, :])

                    pend = []
                    for kt in range(nkt):
                        pend.append(emit_scores(kt))
                        if len(pend) > 2:
                            emit_av(*pend.pop(0))
                    for item in pend:
                        emit_av(*item)

                    ps_den = ps_denp.tile([1, P * G4], f32, tag="ps_den")
                    nc.tensor.matmul(ps_den[:], ones_col[:],
                                     ptil[:].rearrange("p q h -> p (q h)"),
                                     start=True, stop=True)
                    den = small.tile([1, P * G4], f32, tag="den")
                    nc.scalar.activation(den[:], ps_den[:],
                                         mybir.ActivationFunctionType.Copy)
                    nc.vector.reciprocal_approx_fast(den[:], den[:])
                    bc = small.tile([P, P, G4], f32, tag="bc")
                    nc.gpsimd.partition_broadcast(bc[:], den[:])
                    ysl = yhat[:, g * G4:(g + 1) * G4, qt * P:(qt + 1) * P] \
                        .rearrange("d h q -> d q h")
                    nc.vector.tensor_mul(ysl, ps_y[:], bc[:])
                    pump(4)
            for cc in range(NCC):
                pending.append(oproj_part(NQT - 1, cc, 0, H))
            pump(10 ** 6)

    nc.compile()
    return nc


def _rope_tables():
    freqs = 1.0 / (THETA ** (np.arange(0, D, 2, dtype=np.float64) / D))
    ang = np.arange(T, dtype=np.float64)[:, None] * freqs[None, :]
    emb = np.concatenate([ang, ang], axis=-1)          # [T, D]
    return np.cos(emb), np.sin(emb)                    # [T, D] each


def _prep_inputs(x, Wq, Wkv, Wo):
    cos, sin = _rope_tables()
    sgn = np.where(np.arange(D) < D // 2, -1.0, 1.0)   # sign for shifted term
    inv = 1.0 / np.sqrt(D)
    cosT = np.ascontiguousarray(cos.T)                 # [D, T]
    sinTs = np.ascontiguousarray(sin.T) * sgn[:, None]

    # pre-tiled layouts: every DMA reads contiguous per-partition runs
    # wq [p, (h ct d)]: wq[p, h, ct, d] = Wq.T[ct*128+p, h*128+d]
    wq_t = np.ascontiguousarray(
        Wq.T.reshape(16, 128, 16, 128).transpose(1, 2, 0, 3).reshape(128, -1)
    ).astype(ml_dtypes.bfloat16)
    # wkv [p, (ct n)]: wkv[p, ct, n] = Wkv.T[ct*128+p, n]
    wkv_t = np.ascontiguousarray(
        Wkv.T.reshape(16, 128, 1024).transpose(1, 0, 2).reshape(128, -1)
    ).astype(ml_dtypes.bfloat16)
    # wo [p, (cc hh c)]: wo[p, cc, hh, c] = Wo.T[hh*128+p, cc*512+c]
    wo_t = np.ascontiguousarray(
        Wo.T.reshape(16, 128, 4, 512).transpose(1, 2, 0, 3).reshape(128, -1)
    ).astype(ml_dtypes.bfloat16)

    in_maps = []
    for c in range(8):
        b, s = c // 4, c % 4
        rows = np.arange(s, T, 4)
        xq = np.ascontiguousarray(
            x[b][rows, :].T.reshape(16, 128, 512).transpose(1, 0, 2).reshape(128, -1)
        ).astype(ml_dtypes.bfloat16)  # [p, (ct q)]
        ch = np.arange(512 * s, 512 * (s + 1))
        xkv = np.ascontiguousarray(
            x[b][ch, :].T.reshape(16, 128, 512).transpose(1, 0, 2).reshape(128, -1)
        ).astype(ml_dtypes.bfloat16)  # [p, (ct t)] chunk
        cq = np.ascontiguousarray(cosT[:, rows] * inv).astype(ml_dtypes.bfloat16)
        sq = np.ascontiguousarray(sinTs[:, rows] * inv).astype(ml_dtypes.bfloat16)
        # staircase mask, transposed: [k-window j, q i]; visible iff j <= 4i+s
        j = np.arange(TQ)[:, None]
        i = np.arange(P)[None, :]
        mask = np.where(j <= 4 * i + s, 0.0, MASK_VAL).astype(np.float32)
        # pre-tiled [p, (kt q)]: mask_t[p, kt, q] = mask[kt*128+p, q]
        mask = np.ascontiguousarray(
            mask.reshape(4, 128, 128).transpose(1, 0, 2).reshape(128, -1)
        ).astype(ml_dtypes.bfloat16)
        in_maps.append({
            "xq": xq, "xkv": xkv,
            "wq": wq_t, "wkv": wkv_t, "wo": wo_t,
            "cos_q": cq, "sin_q": sq,
            "cos_k": np.ascontiguousarray(cosT[:, ch]).astype(ml_dtypes.bfloat16),
            "sin_k": np.ascontiguousarray(sinTs[:, ch]).astype(ml_dtypes.bfloat16),
            "mask": mask,
        })
    return in_maps


def _unshard(results):
    full = np.empty((B, T, C), dtype=np.float32)
    for c in range(8):
        b, s = c // 4, c % 4
        full[b, s::4, :] = results[c]["out"].astype(np.float32)
    return full


def run(x, Wq, Wkv, Wo, trace=False, trace_kwargs=None):
    import time
    if "nc" not in _compiled:
        _compiled["nc"] = _build()
    nc = _compiled["nc"]
    in_maps = _prep_inputs(np.asarray(x), np.asarray(Wq), np.asarray(Wkv), np.asarray(Wo))
    last_err = None
    for attempt in range(3):
        try:
            res = run_bass_kernel_spmd(nc, in_maps, core_ids=list(range(8)), trace=trace,
                                       **(trace_kwargs or {}))
            return _unshard(res.results), res
        except Exception as e:  # transient NRT device errors recover on retry
            last_err = e
            time.sleep(5)
    raise last_err


def kernel(x, Wq, Wkv, Wo):
    out, _ = run(x, Wq, Wkv, Wo, trace=False)
    return out



# revision 49
# speedup vs baseline: 1.0325x; 1.0325x over previous
"""Distributed GQA attention (B=2,T=2048,C=2048,H=16,KV=4,D=128, RoPE, causal)
for one TRN2 chip (8 NeuronCores).

Sharding (no collectives except KV AllGather): core c -> batch b=c//4,
stripe s=c%4. Each core handles query rows {r : r % 4 == s} of its batch
(512 rows, interleaved so causal spans are shape-uniform across cores ->
one SPMD graph), computes K/V for its 512-token chunk (KV proj sharded,
AllGather within the 4-core batch group), and produces complete output
rows. Host reassembles by stripe.

Scheduling model (from perfetto forensics): ONE dynamic DMA engine
(~200GB/s) serves the sync/scalar/gpsimd queues round-robin and crawls to
~50-70GB/s while the AllGather's CC engines run; the PE sustains ~1 bf16
col / 0.6ns (power-throttle duty ~69%) making total matmul columns the
hard floor (~285us); the AllGather (barrier 15-70us + transfer 42-75us)
is high-variance and must stay off the critical path.

Per-core pipeline:
  PE-warmup dummy matmuls at t~0 (p-state) while the sync queue pulls, in
    need-order: KV operands -> rope-k tables -> xq -> 10 Wq head-tiles.
    Scalar queue stays empty so the incremental K/V staging descriptors
    fire the moment RoPE/V-copy finish (~55us); deferrable loads (rope-q
    tables, mask, late Wq heads) queue behind them to ride the crawl.
  KVproj (ct-outer, K-banks/V-banks in separate PSUM pools so Qproj's
    accumulators reuse the earliest-released banks) -> RoPE on DVE ->
    stage -> AllGather; Wo loads ride gpsimd after the V readbacks into
    the SBUF region freed by the KV pool (no pool-close barrier on the
    Qproj/attention path waits on staging or Wo).
  Qproj overlaps the AllGather; K readbacks on sync / V on gpsimd, rank 0
    first, so attention starts the moment rank-0 data lands.
  attention, scores transposed: S^T[k,(h4,q)] = K-tile^T . Q(4 heads)
    + staircase causal mask (DVE), exp on ACT -> P^T bf16; softmax
    denominator is a bf16 running sum of P^T tiles + ONE ones-vector
    matmul per (qt,g); staircase tiles use partial-free APs.
  Oproj chains are generators pumped one matmul at a time between
    attention matmuls (PE filler while ACT paces the exp chain); outputs
    drain on the (late-idle) sync queue in bf16.
"""

import numpy as np
import ml_dtypes

import concourse.bass as bass
import concourse.tile as tile
from concourse import bacc, mybir
from concourse.bass_utils import run_bass_kernel_spmd

B, T, C = 2, 2048, 2048
H, KV, D = 16, 4, 128
G4 = H // KV            # q heads per kv head
THETA = 10000.0
P = 128
CT = C // P             # 16 c-tiles
TQ = 512                # queries per core
NQT = TQ // P           # 4 q-tiles
NTT = T // P            # 16 token tiles
NCC = C // 512          # 4 output column blocks
MASK_VAL = -1e5

f32 = mybir.dt.float32
bf16 = mybir.dt.bfloat16

_compiled = {}


def _build():
    nc = bacc.Bacc("TRN2", target_bir_lowering=False, debug=False, num_devices=8)
    xq_e = nc.dram_tensor("xq", [P, CT * TQ], bf16, kind="ExternalInput")
    xkv_e = nc.dram_tensor("xkv", [P, CT * TQ], bf16, kind="ExternalInput")  # chunk, pre-tiled [p,(ct t)]
    wq_e = nc.dram_tensor("wq", [P, H * CT * D], bf16, kind="ExternalInput")  # [p,(h ct d)]
    wkv_e = nc.dram_tensor("wkv", [P, CT * 2 * KV * D], bf16, kind="ExternalInput")  # [p,(ct n)]
    wo_e = nc.dram_tensor("wo", [P, NCC * H * 512], bf16, kind="ExternalInput")  # [p,(cc hh c)]
    cq_e = nc.dram_tensor("cos_q", [D, TQ], bf16, kind="ExternalInput")
    sq_e = nc.dram_tensor("sin_q", [D, TQ], bf16, kind="ExternalInput")
    ck_e = nc.dram_tensor("cos_k", [D, TQ], bf16, kind="ExternalInput")  # chunk positions
    sk_e = nc.dram_tensor("sin_k", [D, TQ], bf16, kind="ExternalInput")
    mk_e = nc.dram_tensor("mask", [P, NQT * P], bf16, kind="ExternalInput")
    out_e = nc.dram_tensor("out", [TQ, C], bf16, kind="ExternalOutput")

    NR = 4  # ranks per batch group

    from contextlib import ExitStack  # noqa: F401

    with tile.TileContext(nc) as tc, ExitStack() as top:
        persist = top.enter_context(tc.tile_pool(name="persist", bufs=1))

        # (A warmup barrier collective was tried and removed: the CC stream's
        # ~45us first-collective rendezvous runs concurrently with KVproj
        # regardless, and each extra CC op adds ~10us of stream turnaround
        # that pushed the real AllGather later.)
        mask_t = persist.tile([P, NQT, P], bf16)
        ones_col = persist.tile([P, 1], bf16)
        nc.vector.memset(ones_col[:], 1.0)
        # PE warmup: ~2.5us of dummy matmuls at t~0 (before any DMA lands)
        # so the PE's p-state/clock is up when the first real KV chain start.
        with tc.tile_pool(name="warm", bufs=1) as warmp, \
             tc.tile_pool(name="ps_warm", bufs=1, space="PSUM") as ps_warmp:
            wsb = warmp.tile([P, TQ], bf16)
            nc.vector.memset(wsb[:], 0.0)
            ps_w = ps_warmp.tile([1, TQ], f32)
            for i in range(8):
                nc.tensor.matmul(ps_w[:], ones_col[:], wsb[:],
                                 start=(i == 0), stop=(i == 7))
        # qhat laid out [d, g, q, j] so the scores moving operand (q-major,
        # head-minor) is contiguous.
        qhat = persist.tile([D, KV, TQ, G4], bf16)
        # per-rank K/V tiles: attention q-tile 0 only depends on rank 0's
        # chunk, so it starts as soon as the first readback DMA lands.
        khat_r = [persist.tile([D, KV, TQ], bf16, name=f"khat{r}") for r in range(NR)]
        vsb_r = [persist.tile([P, NQT, KV * D], bf16, name=f"vsb{r}") for r in range(NR)]
        yhat = persist.tile([D, H, TQ], bf16)

        # Q operand pools (DMAs on the scalar queue so the sync queue stays
        # clear for the collective staging + readback). Kept open to the end
        # (LIFO pool order: wop is created after the KV pool closes so it can
        # reuse that region, and must outlive these).
        xqp = top.enter_context(tc.tile_pool(name="xqp", bufs=1))
        tabq = top.enter_context(tc.tile_pool(name="tabq", bufs=1))
        # Wq stream pool coexists with the KV pool: 9 head-tiles prefetch
        # with no SBUF anti-dependency, deep enough that Qproj never starves
        # while the AllGather window throttles concurrent DMA to ~60GB/s.
        wstream = top.enter_context(tc.tile_pool(name="wqstream", bufs=9))

        # ---- KV chunk projection (bf16) + AllGather ---------------------
        with tc.tile_pool(name="kvchunk", bufs=1) as kvc, \
             tc.tile_pool(name="dram", bufs=1, space="DRAM") as dram, \
             tc.tile_pool(name="ps_kvk", bufs=1, space="PSUM") as ps_kvk, \
             tc.tile_pool(name="ps_kvv", bufs=1, space="PSUM") as ps_kvv, \
             tc.tile_pool(name="ropek", bufs=1) as ropekp:
            # per-c-tile tiles so the first KV matmuls start as soon as the
            # first chunks land (exact per-tile deps). ct-outer loop with all
            # 8 K/V accumulators resident (8 PSUM banks) -> the tensor engine
            # starts ~1us in and streams DMA-paced, instead of waiting for
            # the full x-chunk before the first chain.
            # 4-ct granularity: 8 big DMAs instead of 32 small ones — the SP
            # sequencer's ~0.6us per-issue cost was pacing the startup.
            xkv_r = xkv_e.ap().rearrange("p (cg ct t) -> p cg ct t", cg=4, ct=4)
            wkv_r = wkv_e.ap().rearrange("p (cg ct n) -> p cg ct n", cg=4, ct=4)
            # first 4 c-tiles as separate small tiles (deps are per-tile, so
            # the first chains start ~1us in); the rest in 512KB groups.
            # Weight tile first within each pair (LDWEIGHTS consumes it first).
            x0s, w0s = [], []
            for ci in range(4):
                wt = kvc.tile([P, 2 * KV * D], bf16, name=f"wkv0_{ci}")
                nc.sync.dma_start(wt[:], wkv_r[:, 0, ci])
                w0s.append(wt)
                xt = kvc.tile([P, TQ], bf16, name=f"xkv0_{ci}")
                nc.sync.dma_start(xt[:], xkv_r[:, 0, ci])
                x0s.append(xt)
            # All KV operands land BEFORE xq: KVproj-end gates the staging ->
            # AllGather -> readback chain (the critical path to attention),
            # while Qproj only starts once the KV chains free the PE (~46us),
            # well after even a late xq.
            xkv4, wkv4 = [None], [None]
            for cg in range(1, 4):
                wt = kvc.tile([P, 4, 2 * KV * D], bf16, name=f"wkv{cg}")
                nc.sync.dma_start(wt[:], wkv_r[:, cg])
                wkv4.append(wt)
                xt = kvc.tile([P, 4, TQ], bf16, name=f"xkv{cg}")
                nc.sync.dma_start(xt[:], xkv_r[:, cg])
                xkv4.append(xt)
            # xq + the first 8 Wq head-tiles follow on the SAME sync queue:
            # one queue pulling serially gets full HBM rate, so the KV
            # operands (which gate staging -> AllGather, the critical path)
            # land first and fast; xq/wq ride the remaining pre-staging window
            # (staging waits on the K RoPE ~50us anyway). The late wq tiles
            # (h8-15) stream on the scalar queue during the AllGather window.
            # ONE DMA engine (~200GB/s) serves all three dynamic queues
            # round-robin, and it crawls (~50GB/s) while the AllGather's CC
            # engines run. So: every byte needed before/at the AllGather is
            # serialized on the sync queue in need-order (KV -> rope-k ->
            # xq -> wq h0..h7), the scalar queue stays EMPTY so the staging
            # descriptors sit at its head and fire the moment RoPE finishes,
            # and everything deferrable (mask, rope-q tables, wq h8-15)
            # queues on scalar BEHIND staging to ride the crawl window.
            cos_k = kvc.tile([D, TQ], bf16)
            nc.sync.dma_start(cos_k[:], ck_e.ap())
            sin_k = kvc.tile([D, TQ], bf16)
            nc.sync.dma_start(sin_k[:], sk_e.ap())
            xq = xqp.tile([P, CT, TQ], bf16)
            xq_r = xq_e.ap().rearrange("p (cg ct q) -> p cg ct q", cg=4, ct=4)
            for cg in range(4):
                nc.sync.dma_start(xq[:, 4 * cg:4 * cg + 4, :], xq_r[:, cg])
            wq_rr = wq_e.ap().rearrange("p (h ct d) -> p h ct d", h=H, ct=CT)
            wq_pre = []
            for h in range(10):
                wqt = wstream.tile([P, CT, D], bf16, tag="wq")
                nc.sync.dma_start(wqt[:], wq_rr[:, h])
                wq_pre.append(wqt)

            def xop(ct, lo=0, hi=TQ):
                if ct < 4:
                    return x0s[ct][:, lo:hi]
                return xkv4[ct // 4][:, ct % 4, lo:hi]

            def wop(ct, lo, hi):
                if ct < 4:
                    return w0s[ct][:, lo:hi]
                return wkv4[ct // 4][:, ct % 4, lo:hi]
            # kchunk/vchunk live in persist: staging reads them, and if they
            # were kvc tiles the kvc pool-close barrier (region reused by
            # later pools) would gate Qproj's start on staging completion.
            kchunk = persist.tile([D, KV, TQ], bf16)
            vchunk = persist.tile([P, NQT, KV * D], bf16)

            # ct-outer with all 8 K/V accumulators resident (8 PSUM banks):
            # every chain progresses with the DMA stream and they all finish
            # right as the last c-tile lands. (g-outer K with trailing ropes
            # measured worse: it delays V/staging and Qproj more than the
            # earlier kchunk helps.)
            # K accumulators in the lower PSUM pool, V in the upper: the V
            # pool closes first (LIFO) so Qproj's ps_q banks reuse the V
            # banks, whose consumers (ACT V-copies, ~50us) finish before the
            # rope chain (~55us) that holds the K banks.
            ps_ks = [ps_kvk.tile([P, TQ], f32, name=f"ps_k{g}") for g in range(KV)]
            ps_vs = [ps_kvv.tile([P, KV * D], f32, name=f"ps_v{t}") for t in range(NQT)]
            for ct in range(CT):
                for g in range(KV):
                    nc.tensor.matmul(ps_ks[g][:],
                                     wop(ct, g * D, (g + 1) * D),
                                     xop(ct),
                                     start=(ct == 0), stop=(ct == CT - 1))
                for ttl in range(NQT):
                    nc.tensor.matmul(ps_vs[ttl][:],
                                     xop(ct, ttl * P, (ttl + 1) * P),
                                     wop(ct, KV * D, 2 * KV * D),
                                     start=(ct == 0), stop=(ct == CT - 1))
            # RoPE on DVE; V drains on ACT in parallel. Stage each K group /
            # V tile to DRAM the moment it's ready (incremental staging pulls
            # the AllGather's data-ready point a few us earlier).
            cc_in = dram.tile([2, P, KV, TQ], bf16)
            cc_out = dram.tile([NR, 2, P, KV, TQ], bf16)
            for g in range(KV):
                ps = ps_ks[g]
                tmp = ropekp.tile([D, TQ], bf16, tag="rope_k")
                nc.vector.tensor_copy(tmp[0:64, :], ps[64:128, :])
                nc.vector.tensor_copy(tmp[64:128, :], ps[0:64, :])
                ksl = kchunk[:, g, :]
                nc.vector.tensor_mul(ksl, ps[:], cos_k[:])
                nc.vector.tensor_mul(tmp[:], tmp[:], sin_k[:])
                nc.vector.tensor_add(ksl, ksl, tmp[:])
                nc.scalar.dma_start(cc_in[0, :, g, :], ksl)
            for ttl in range(NQT):
                nc.scalar.activation(vchunk[:, ttl, :], ps_vs[ttl][:],
                                     mybir.ActivationFunctionType.Copy)
                nc.scalar.dma_start(
                    cc_in[1].rearrange("p g x -> p (g x)")[:, ttl * KV * D:(ttl + 1) * KV * D],
                    vchunk[:, ttl, :])
            # deferrable loads queue on scalar BEHIND staging: rope-q tables
            # (the first Qproj rope may lag; attention only needs qhat at
            # ~137us) and the mask (needed at attention start).
            cos_q = tabq.tile([D, TQ], bf16)
            nc.scalar.dma_start(cos_q[:], cq_e.ap())
            sin_q = tabq.tile([D, TQ], bf16)
            nc.scalar.dma_start(sin_q[:], sq_e.ap())
            nc.scalar.dma_start(mask_t[:], mk_e.ap().rearrange("p (kt q) -> p kt q", kt=NQT))
            nc.gpsimd.collective_compute(
                "AllGather",
                mybir.AluOpType.bypass,
                replica_groups=[[0, 1, 2, 3], [4, 5, 6, 7]],
                ins=[cc_in[:].opt()],
                outs=[cc_out[:].opt()],
            )
            # per-rank readbacks, rank 0 first: attention starts on rank 0.
            # K on the sync queue, V on the gpsimd queue -> ~2x readback rate.
            for r in range(NR):
                nc.sync.dma_start(khat_r[r][:], cc_out[r, 0])
                nc.gpsimd.dma_start(
                    vsb_r[r][:].rearrange("p ttl n -> p (ttl n)"),
                    cc_out[r, 1].rearrange("p g x -> p (g x)"))

        # ---- Wo resident: loaded right after the KV section closes so the
        # allocator reuses the freed kvc region (anti-dep ~55us, not Qproj
        # end), on the gpsimd queue behind the V readbacks (lands ~150us,
        # needed ~190us) so no sync/scalar semaphore threshold that gates
        # attention's first matmuls ever covers these 4MB.
        wop = top.enter_context(tc.tile_pool(name="wop", bufs=1))
        wo_r = wo_e.ap().rearrange("p (cc hh c) -> p cc hh c", cc=NCC, hh=H)
        wos = []
        for cc in range(NCC):
            wt = wop.tile([P, H, 512], bf16, name=f"wo{cc}")
            nc.gpsimd.dma_start(wt[:], wo_r[:, cc])
            wos.append(wt)

        # ---- Q projection (bf16, weights streamed via scalar queue) -----
        with tc.tile_pool(name="ps_qp", bufs=3, space="PSUM") as ps_q, \
             tc.tile_pool(name="ropeq", bufs=1) as ropep:
            for h in range(H):
                if h < 10:
                    wqt = wq_pre[h]
                else:
                    wqt = wstream.tile([P, CT, D], bf16, tag="wq")
                    nc.scalar.dma_start(
                        wqt[:], wq_e.ap().rearrange("p (h ct d) -> p h ct d", h=H, ct=CT)[:, h])
                ps = ps_q.tile([P, TQ], f32, tag="ps_q")
                for ct in range(CT):
                    nc.tensor.matmul(ps[:], wqt[:, ct, :], xq[:, ct, :],
                                     start=(ct == 0), stop=(ct == CT - 1))
                tmp = ropep.tile([D, TQ], bf16, tag="rope_q")
                nc.vector.tensor_copy(tmp[0:64, :], ps[64:128, :])
                nc.vector.tensor_copy(tmp[64:128, :], ps[0:64, :])
                qsl = qhat[:, h // G4, :, h % G4]
                nc.vector.tensor_mul(qsl, ps[:], cos_q[:])
                nc.vector.tensor_mul(tmp[:], tmp[:], sin_q[:])
                nc.vector.tensor_add(qsl, qsl, tmp[:])

        # ---- attention (scores transposed) + interleaved Oproj ----------
        with tc.tile_pool(name="ptile", bufs=5) as ptp, \
             tc.tile_pool(name="ptil", bufs=2) as ptilp, \
             tc.tile_pool(name="small", bufs=2) as small, \
             tc.tile_pool(name="outp", bufs=1) as outp, \
             tc.tile_pool(name="ps_s", bufs=3, space="PSUM") as ps_sp, \
             tc.tile_pool(name="ps_y", bufs=2, space="PSUM") as ps_yp, \
             tc.tile_pool(name="ps_den", bufs=1, space="PSUM") as ps_denp, \
             tc.tile_pool(name="ps_o", bufs=2, space="PSUM") as ps_op:
            # Oproj chains are emitted as generators and "pumped" one matmul
            # at a time between attention matmuls: the PE gets filler work
            # whenever the exp chain would otherwise let it idle (keeping the
            # p-state up), and the Oproj phase all but disappears.
            from collections import deque
            pending = deque()

            # Oproj chains are split: heads 0-11 (groups g0-g2) become
            # pump-able right after g2's y-slice, so the PE has filler DURING
            # the same q-tile's last (biggest) group instead of exhausting the
            # previous q-tile's chains early; heads 12-15 follow after g3.
            ps_o_map = {}

            def oproj_part(qt, cc, h_lo, h_hi):
                if (qt, cc) not in ps_o_map:
                    ps_o_map[(qt, cc)] = ps_op.tile([P, 512], f32, tag="ps_o",
                                                    name=f"ps_o_{qt}_{cc}")
                ps_o = ps_o_map[(qt, cc)]
                for hh in range(h_lo, h_hi):
                    nc.tensor.matmul(ps_o[:], yhat[:, hh, qt * P:(qt + 1) * P],
                                     wos[cc][:, hh, :],
                                     start=(hh == 0), stop=(hh == H - 1))
                    yield
                if h_hi == H:
                    ps_o_map.pop((qt, cc))
                    osb = outp.tile([P, 512], bf16, tag="osb")
                    nc.scalar.activation(osb[:], ps_o[:],
                                         mybir.ActivationFunctionType.Copy)
                    # sync queue: idle after the K readbacks, and the SP
                    # engine fires triggers promptly (gpsimd's serialize
                    # behind partition_broadcast, delaying the final drain).
                    nc.sync.dma_start(
                        out_e.ap()[qt * P:(qt + 1) * P, cc * 512:(cc + 1) * 512], osb[:])

            def pump(n):
                for _ in range(n):
                    if not pending:
                        return
                    try:
                        next(pending[0])
                    except StopIteration:
                        pending.popleft()

            for qt in range(NQT):
                nkt = 4 * (qt + 1)
                if qt > 0:
                    for cc in range(NCC):
                        pending.append(oproj_part(qt - 1, cc, 0, H))
                for g in range(KV):
                    # free layout is (q, h4) so staircase partial slices
                    # (q >= q0) are contiguous prefixes -> 2-dim APs.
                    ps_y = ps_yp.tile([P, P, G4], f32, tag="ps_y")
                    ptil = ptilp.tile([P, P, G4], bf16, tag="ptil")

                    def emit_scores(kt):
                        ktl = kt - (nkt - 4)  # staircase index when >= 0
                        q0 = 32 * ktl if ktl > 0 else 0
                        ps_s = ps_sp.tile([P, P, G4], f32, tag="ps_s")
                        nc.tensor.matmul(
                            ps_s[:, q0:, :],
                            khat_r[kt // 4][:, g, (kt % 4) * P:(kt % 4 + 1) * P],
                            qhat[:, g, qt * P + q0:(qt + 1) * P, :],
                            start=True, stop=True)
                        if ktl >= 0:
                            nc.vector.tensor_add(
                                ps_s[:, q0:, :], ps_s[:, q0:, :],
                                mask_t[:, ktl, q0:, None].to_broadcast((P, P - q0, G4)))
                        pt = ptp.tile([P, P, G4], bf16, tag="pt")
                        nc.scalar.activation(pt[:, q0:, :], ps_s[:, q0:, :],
                                             mybir.ActivationFunctionType.Exp)
                        return (kt, q0, pt)

                    def emit_av(kt, q0, pt):
                        nc.tensor.matmul(ps_y[:, q0:, :],
                                         vsb_r[kt // 4][:, kt % 4, g * D:(g + 1) * D],
                                         pt[:, q0:, :],
                                         start=(kt == 0), stop=(kt == nkt - 1))
                        pump(2)
                        # P-tile running sum (softmax denominator), partial
                        # slices on staircase tiles (the skipped prefix is
                        # exactly-masked, i.e. contributes zero), emitted
                        # trailing the exp by the AV lookahead so DVE's
                        # in-order stream never blocks a mask add on an exp.
                        if kt == 0:
                            nc.vector.tensor_copy(ptil[:], pt[:])
                        else:
                            nc.vector.tensor_add(ptil[:, q0:, :], ptil[:, q0:, :],
                                                 pt[:, q0:, :])

                    pend = []
                    for kt in range(nkt):
                        pend.append(emit_scores(kt))
                        if len(pend) > 2:
                            emit_av(*pend.pop(0))
                    for item in pend:
                        emit_av(*item)

                    ps_den = ps_denp.tile([1, P * G4], f32, tag="ps_den")
                    nc.tensor.matmul(ps_den[:], ones_col[:],
                                     ptil[:].rearrange("p q h -> p (q h)"),
                                     start=True, stop=True)
                    den = small.tile([1, P * G4], f32, tag="den")
                    nc.scalar.activation(den[:], ps_den[:],
                                         mybir.ActivationFunctionType.Copy)
                    nc.vector.reciprocal_approx_fast(den[:], den[:])
                    bc = small.tile([P, P, G4], f32, tag="bc")
                    nc.gpsimd.partition_broadcast(bc[:], den[:])
                    ysl = yhat[:, g * G4:(g + 1) * G4, qt * P:(qt + 1) * P] \
                        .rearrange("d h q -> d q h")
                    nc.vector.tensor_mul(ysl, ps_y[:], bc[:])
                    pump(4)
            for cc in range(NCC):
                pending.append(oproj_part(NQT - 1, cc, 0, H))
            pump(10 ** 6)

    nc.compile()
    return nc


def _rope_tables():
    freqs = 1.0 / (THETA ** (np.arange(0, D, 2, dtype=np.float64) / D))
    ang = np.arange(T, dtype=np.float64)[:, None] * freqs[None, :]
    emb = np.concatenate([ang, ang], axis=-1)          # [T, D]
    return np.cos(emb), np.sin(emb)                    # [T, D] each


def _prep_inputs(x, Wq, Wkv, Wo):
    cos, sin = _rope_tables()
    sgn = np.where(np.arange(D) < D // 2, -1.0, 1.0)   # sign for shifted term
    inv = 1.0 / np.sqrt(D)
    cosT = np.ascontiguousarray(cos.T)                 # [D, T]
    sinTs = np.ascontiguousarray(sin.T) * sgn[:, None]

    # pre-tiled layouts: every DMA reads contiguous per-partition runs
    # wq [p, (h ct d)]: wq[p, h, ct, d] = Wq.T[ct*128+p, h*128+d]
    wq_t = np.ascontiguousarray(
        Wq.T.reshape(16, 128, 16, 128).transpose(1, 2, 0, 3).reshape(128, -1)
    ).astype(ml_dtypes.bfloat16)
    # wkv [p, (ct n)]: wkv[p, ct, n] = Wkv.T[ct*128+p, n]
    wkv_t = np.ascontiguousarray(
        Wkv.T.reshape(16, 128, 1024).transpose(1, 0, 2).reshape(128, -1)
    ).astype(ml_dtypes.bfloat16)
    # wo [p, (cc hh c)]: wo[p, cc, hh, c] = Wo.T[hh*128+p, cc*512+c]
    wo_t = np.ascontiguousarray(
        Wo.T.reshape(16, 128, 4, 512).transpose(1, 2, 0, 3).reshape(128, -1)
    ).astype(ml_dtypes.bfloat16)

    in_maps = []
    for c in range(8):
        b, s = c // 4, c % 4
        rows = np.arange(s, T, 4)
        xq = np.ascontiguousarray(
            x[b][rows, :].T.reshape(16, 128, 512).transpose(1, 0, 2).reshape(128, -1)
        ).astype(ml_dtypes.bfloat16)  # [p, (ct q)]
        ch = np.arange(512 * s, 512 * (s + 1))
        xkv = np.ascontiguousarray(
            x[b][ch, :].T.reshape(16, 128, 512).transpose(1, 0, 2).reshape(128, -1)
        ).astype(ml_dtypes.bfloat16)  # [p, (ct t)] chunk
        cq = np.ascontiguousarray(cosT[:, rows] * inv).astype(ml_dtypes.bfloat16)
        sq = np.ascontiguousarray(sinTs[:, rows] * inv).astype(ml_dtypes.bfloat16)
        # staircase mask, transposed: [k-window j, q i]; visible iff j <= 4i+s
        j = np.arange(TQ)[:, None]
        i = np.arange(P)[None, :]
        mask = np.where(j <= 4 * i + s, 0.0, MASK_VAL).astype(np.float32)
        # pre-tiled [p, (kt q)]: mask_t[p, kt, q] = mask[kt*128+p, q]
        mask = np.ascontiguousarray(
            mask.reshape(4, 128, 128).transpose(1, 0, 2).reshape(128, -1)
        ).astype(ml_dtypes.bfloat16)
        in_maps.append({
            "xq": xq, "xkv": xkv,
            "wq": wq_t, "wkv": wkv_t, "wo": wo_t,
            "cos_q": cq, "sin_q": sq,
            "cos_k": np.ascontiguousarray(cosT[:, ch]).astype(ml_dtypes.bfloat16),
            "sin_k": np.ascontiguousarray(sinTs[:, ch]).astype(ml_dtypes.bfloat16),
            "mask": mask,
        })
    return in_maps


def _unshard(results):
    full = np.empty((B, T, C), dtype=np.float32)
    for c in range(8):
        b, s = c // 4, c % 4
        full[b, s::4, :] = results[c]["out"].astype(np.float32)
    return full


def run(x, Wq, Wkv, Wo, trace=False, trace_kwargs=None):
    import time
    if "nc" not in _compiled:
        _compiled["nc"] = _build()
    nc = _compiled["nc"]
    in_maps = _prep_inputs(np.asarray(x), np.asarray(Wq), np.asarray(Wkv), np.asarray(Wo))
    last_err = None
    for attempt in range(3):
        try:
            res = run_bass_kernel_spmd(nc, in_maps, core_ids=list(range(8)), trace=trace,
                                       **(trace_kwargs or {}))
            return _unshard(res.results), res
        except Exception as e:  # transient NRT device errors recover on retry
            last_err = e
            time.sleep(5)
    raise last_err


def kernel(x, Wq, Wkv, Wo):
    out, _ = run(x, Wq, Wkv, Wo, trace=False)
    return out

